# revision 17
# baseline (speedup 1.0000x reference)
"""AGNNConv on 8 TRN2 NeuronCores — pure-compute streaming design.

This platform (axon/PJRT TRN2) has no usable data-dependent DMA: the
custom SWDGE gather/scatter ucode crashes the device and the generic
indirect DMA path is a ~66us/call software queue.  So the kernel is
built exclusively from streaming DMA + compute engines:

  - Host (pure layout, no numerics): partition edges by dst window of
    128 nodes (98 windows per core, window-aligned core ranges of 12544
    nodes), pad each window's edge list to T tiles of 128 edge slots,
    and materialize per-edge operand rows fs = feat[src], fd = feat[dst]
    in the exact SBUF layout the device consumes ("node features
    replicated per edge" — the extreme of the sharding hint).
  - Device per window batch:
      ss_s, ss_d row sums of squares (raw rows -> norms, same math as
      reference), cos = sum(fs*fd) * rsqrt(max(ss_s*ss_d, eps)),
      p = exp(beta*cos)           (softmax max-subtraction dropped:
                                   |beta*cos| <= |beta|, well-conditioned,
                                   mathematically identical)
      payload = [p*fs | p] bf16
      scatter: per 128-edge tile a one-hot matrix A[e, m] =
      (dst%128 == m) built on DVE, and PE matmul A^T @ payload
      accumulates [128 nodes, 33] in PSUM across the window's tiles.
      out = msg / s on evacuation.
  - Pad edges get dst8 = 255 -> all-zero one-hot row -> contribute
    nothing.  Zero fs/fd pad rows stay finite through the norm chain.
"""

import sys

if "/opt/trn_rl_repo" not in sys.path:
    sys.path.insert(0, "/opt/trn_rl_repo")

import numpy as np

# Problem constants (hardcoded per harness contract)
N_NODES = 100000
N_EDGES = 1600000
D = 32
NCORES = 8
NW = 98            # dst windows (128 nodes) per core
NLOC = NW * 128    # 12544 nodes per core (window-aligned; trimmed on host)
TPW = 18           # tiles (128 edge slots) per window; cap 2304 >= max 2191
WB = 2             # windows per compute batch (98 = 49*2)
PW = D + 1         # payload width


def build_graph(nw, tpw, wb, d=D, repeat=1):
    import concourse.bass as bass
    import concourse.tile as tile
    from concourse import bacc, mybir
    from contextlib import nullcontext

    f32 = mybir.dt.float32
    bf16 = mybir.dt.bfloat16
    X = mybir.AxisListType.X
    ADD = mybir.AluOpType.add
    ISEQ = mybir.AluOpType.is_equal

    assert nw % wb == 0
    nb = nw // wb
    tb = wb * tpw  # tiles per batch

    nc = bacc.Bacc(None, target_bir_lowering=False, debug=False)
    fs_p = nc.declare_dram_parameter("fs", [nw, 128, tpw, d], f32, isOutput=False)
    fd_p = nc.declare_dram_parameter("fd", [nw, 128, tpw, d], f32, isOutput=False)
    d8_p = nc.declare_dram_parameter("d8", [nw, 128, tpw], bf16, isOutput=False)
    iota_p = nc.declare_dram_parameter("iota", [1, 1, 128], bf16, isOutput=False)
    beta_p = nc.declare_dram_parameter("beta", [1], f32, isOutput=False)
    out_p = nc.declare_dram_parameter("out", [nw * 128, d], f32, isOutput=True)

    outR = out_p[:].rearrange("(w m) c -> m w c", m=128)

    with tile.TileContext(nc) as tc:
        with tc.tile_pool(name="singles", bufs=1) as singles:
            beta_sb = singles.tile([128, 1], f32)
            nc.sync.dma_start(out=beta_sb[:], in_=beta_p[:].to_broadcast([128, 1]))
            iota_sb = singles.tile([128, 1, 128], bf16)
            nc.sync.dma_start(
                out=iota_sb[:], in_=iota_p[:].to_broadcast([128, 1, 128])
            )
            obuf = singles.tile([128, nw, d], f32)

            with (
                tc.tile_pool(name="inp", bufs=3) as inp,
                tc.tile_pool(name="ap_", bufs=2) as ap_,
                tc.tile_pool(name="med", bufs=2) as med,
                tc.tile_pool(name="sml", bufs=3) as sml,
                tc.tile_pool(name="ps_", bufs=4, space="PSUM") as ps_,
                tc.For_i(0, repeat, 1) if repeat > 1 else nullcontext(),
            ):
                for b in range(nb):
                    ws = slice(b * wb, (b + 1) * wb)
                    fs_t = inp.tile([128, wb, tpw, d], f32)
                    nc.sync.dma_start(
                        out=fs_t[:], in_=fs_p[ws].rearrange("w i t c -> i w t c")
                    )
                    fd_t = inp.tile([128, wb, tpw, d], f32)
                    nc.sync.dma_start(
                        out=fd_t[:], in_=fd_p[ws].rearrange("w i t c -> i w t c")
                    )
                    d8_t = inp.tile([128, wb, tpw, 1], bf16)
                    nc.sync.dma_start(
                        out=d8_t[:],
                        in_=d8_p[ws].rearrange("w i t -> i w t")[:, :, :, None],
                    )

                    # one-hot A[e-partition, m] per tile (bf16, exact 0/1)
                    A_t = ap_.tile([128, tb, 128], bf16)
                    nc.vector.tensor_tensor(
                        out=A_t[:],
                        in0=d8_t[:].rearrange("i w t o -> i (w t) o").to_broadcast(
                            [128, tb, 128]
                        ),
                        in1=iota_sb[:].to_broadcast([128, tb, 128]),
                        op=ISEQ,
                    )

                    fsf = fs_t[:].rearrange("i w t c -> i (w t) c")
                    fdf = fd_t[:].rearrange("i w t c -> i (w t) c")

                    # cos numerator (DVE)
                    prod = med.tile([128, tb, d], f32)
                    nc.vector.tensor_mul(prod[:], fsf, fdf)
                    cos = sml.tile([128, tb, 1], f32)
                    nc.vector.tensor_reduce(cos[:], prod[:], axis=X, op=ADD)
                    # squared norms (Pool engine)
                    sq = med.tile([128, tb, d], f32)
                    nc.gpsimd.tensor_mul(sq[:], fsf, fsf)
                    ss_s = sml.tile([128, tb, 1], f32)
                    nc.vector.tensor_reduce(ss_s[:], sq[:], axis=X, op=ADD)
                    sq2 = med.tile([128, tb, d], f32)
                    nc.gpsimd.tensor_mul(sq2[:], fdf, fdf)
                    ss_d = sml.tile([128, tb, 1], f32)
                    nc.vector.tensor_reduce(ss_d[:], sq2[:], axis=X, op=ADD)

                    # rn = 1/sqrt(max(ss_s*ss_d, 1e-24)); cn = cos*rn
                    ssp = sml.tile([128, tb, 1], f32)
                    nc.vector.tensor_mul(ssp[:], ss_s[:], ss_d[:])
                    nc.vector.tensor_scalar_max(ssp[:], ssp[:], 1e-24)
                    nrm = sml.tile([128, tb, 1], f32)
                    nc.scalar.sqrt(nrm[:], ssp[:])
                    rn = sml.tile([128, tb, 1], f32)
                    nc.vector.reciprocal(rn[:], nrm[:])
                    cn = sml.tile([128, tb, 1], f32)
                    nc.vector.tensor_mul(cn[:], cos[:], rn[:])
                    # p = exp(beta*cn)
                    p_t = sml.tile([128, tb, 1], f32)
                    nc.scalar.activation(
                        p_t[:], cn[:], mybir.ActivationFunctionType.Exp,
                        scale=beta_sb[:],
                    )

                    # payload [p*fs | p] in bf16
                    pay = med.tile([128, tb, PW], bf16)
                    nc.vector.tensor_mul(
                        pay[:, :, 0:d], fsf, p_t[:].to_broadcast([128, tb, d])
                    )
                    nc.vector.tensor_copy(out=pay[:, :, d : d + 1], in_=p_t[:])

                    # scatter: per-window PSUM accumulation over tiles
                    for wj in range(wb):
                        ps = ps_.tile([128, PW], f32)
                        for t in range(tpw):
                            ti = wj * tpw + t
                            nc.tensor.matmul(
                                ps[:],
                                lhsT=A_t[:, ti, :],
                                rhs=pay[:, ti, :],
                                start=(t == 0),
                                stop=(t == tpw - 1),
                            )
                        w = b * wb + wj
                        sc = sml.tile([128, 1], f32)
                        nc.vector.tensor_scalar_max(sc[:], ps[:, d : d + 1], 1e-30)
                        rc = sml.tile([128, 1], f32)
                        nc.vector.reciprocal(rc[:], sc[:])
                        nc.vector.tensor_mul(
                            obuf[:, w, :], ps[:, 0:d], rc[:].to_broadcast([128, d])
                        )

            nc.sync.dma_start(out=outR[:, :, :], in_=obuf[:])

    nc.compile()
    return nc


def host_prep(feat, beta, src, dst, ncores, nw, tpw, d):
    """Pure index/layout prep. Returns per-core input maps."""
    feat = np.ascontiguousarray(np.asarray(feat, dtype=np.float32))
    beta = np.ascontiguousarray(np.asarray(beta, dtype=np.float32))
    src = np.asarray(src).astype(np.int64)
    dst = np.asarray(dst).astype(np.int64)
    n_nodes = feat.shape[0]
    nloc = nw * 128
    cap = tpw * 128

    win = dst // 128                       # global window id
    order = np.argsort(win, kind="stable")
    src_s, dst_s = src[order], dst[order]
    win_s = win[order]
    # rank of each edge within its window
    wcnt = np.bincount(win_s, minlength=ncores * nw)
    assert wcnt.max() <= cap, f"window overflow: {wcnt.max()} > {cap}"
    starts = np.concatenate([[0], np.cumsum(wcnt)[:-1]])
    rank = np.arange(src_s.size) - starts[win_s]

    iota = np.arange(128, dtype=np.float32)[None, None, :]

    in_maps = []
    for c in range(ncores):
        lo_w, hi_w = c * nw, (c + 1) * nw
        sel = (win_s >= lo_w) & (win_s < hi_w)
        e_src, e_dst, e_win, e_rank = (
            src_s[sel], dst_s[sel], win_s[sel] - lo_w, rank[sel],
        )
        t_ = e_rank // 128
        i_ = e_rank % 128

        fs = np.zeros((nw, 128, tpw, d), dtype=np.float32)
        fd = np.zeros((nw, 128, tpw, d), dtype=np.float32)
        d8 = np.full((nw, 128, tpw), 255.0, dtype=np.float32)
        fs[e_win, i_, t_] = feat[e_src]
        fd[e_win, i_, t_] = feat[e_dst]
        d8[e_win, i_, t_] = (e_dst % 128).astype(np.float32)

        in_maps.append(
            {
                "fs": fs,
                "fd": fd,
                "d8": d8,  # f32 values, cast below
                "iota": iota,
                "beta": beta,
            }
        )
    # bf16 conversion via uint32 view trick (values are small ints: exact)
    import ml_dtypes

    for m in in_maps:
        m["d8"] = m["d8"].astype(ml_dtypes.bfloat16)
        m["iota"] = m["iota"].astype(ml_dtypes.bfloat16)
    return in_maps


_CACHED = {}


def kernel(feat, beta, src, dst):
    from concourse.bass_utils import run_bass_kernel_spmd

    in_maps = host_prep(feat, beta, src, dst, NCORES, NW, TPW, D)
    if "nc" not in _CACHED:
        _CACHED["nc"] = build_graph(NW, TPW, WB)
    nc = _CACHED["nc"]
    res = run_bass_kernel_spmd(nc, in_maps, list(range(NCORES))).results
    full = np.concatenate([res[c]["out"] for c in range(NCORES)], axis=0)
    return full[:N_NODES].astype(np.float32)


# revision 18
# speedup vs baseline: 1.4413x; 1.4413x over previous
"""AGNNConv on 8 TRN2 NeuronCores — pure-compute streaming design.

This platform (axon/PJRT TRN2) has no usable data-dependent DMA: the
custom SWDGE gather/scatter ucode crashes the device and the generic
indirect DMA path is a ~66us/call software queue.  So the kernel is
built exclusively from streaming DMA + compute engines:

  - Host (pure layout, no numerics): partition edges by dst window of
    128 nodes (98 windows per core, window-aligned core ranges of 12544
    nodes), pad each window's edge list to T tiles of 128 edge slots,
    and materialize per-edge operand rows fs = feat[src], fd = feat[dst]
    in the exact SBUF layout the device consumes ("node features
    replicated per edge" — the extreme of the sharding hint).
  - Device per window batch:
      ss_s, ss_d row sums of squares (raw rows -> norms, same math as
      reference), cos = sum(fs*fd) * rsqrt(max(ss_s*ss_d, eps)),
      p = exp(beta*cos)           (softmax max-subtraction dropped:
                                   |beta*cos| <= |beta|, well-conditioned,
                                   mathematically identical)
      payload = [p*fs | p] bf16
      scatter: per 128-edge tile a one-hot matrix A[e, m] =
      (dst%128 == m) built on DVE, and PE matmul A^T @ payload
      accumulates [128 nodes, 33] in PSUM across the window's tiles.
      out = msg / s on evacuation.
  - Pad edges get dst8 = 255 -> all-zero one-hot row -> contribute
    nothing.  Zero fs/fd pad rows stay finite through the norm chain.
"""

import sys

if "/opt/trn_rl_repo" not in sys.path:
    sys.path.insert(0, "/opt/trn_rl_repo")

import numpy as np

# Problem constants (hardcoded per harness contract)
N_NODES = 100000
N_EDGES = 1600000
D = 32
NCORES = 8
NW = 98            # dst windows (128 nodes) per core
NLOC = NW * 128    # 12544 nodes per core (window-aligned; trimmed on host)
TPW = 18           # tiles (128 edge slots) per window; cap 2304 >= max 2191
WB = 7             # windows per compute batch (98 = 14*7)
PW = D + 1         # payload width


def build_graph(nw, tpw, wb, d=D, repeat=1):
    import concourse.bass as bass
    import concourse.tile as tile
    from concourse import bacc, mybir
    from contextlib import nullcontext

    f32 = mybir.dt.float32
    bf16 = mybir.dt.bfloat16
    X = mybir.AxisListType.X
    ADD = mybir.AluOpType.add
    ISEQ = mybir.AluOpType.is_equal

    assert nw % wb == 0
    nb = nw // wb
    tb = wb * tpw  # tiles per batch

    nc = bacc.Bacc(None, target_bir_lowering=False, debug=False)
    fs_p = nc.declare_dram_parameter("fs", [nw, 128, tpw, d], bf16, isOutput=False)
    fd_p = nc.declare_dram_parameter("fd", [nw, 128, tpw, d], bf16, isOutput=False)
    d8_p = nc.declare_dram_parameter("d8", [nw, 128, tpw], bf16, isOutput=False)
    iota_p = nc.declare_dram_parameter("iota", [1, 1, 128], bf16, isOutput=False)
    beta_p = nc.declare_dram_parameter("beta", [1], f32, isOutput=False)
    out_p = nc.declare_dram_parameter("out", [nw * 128, d], f32, isOutput=True)

    outR = out_p[:].rearrange("(w m) c -> m w c", m=128)

    with tile.TileContext(nc) as tc:
        with tc.tile_pool(name="singles", bufs=1) as singles:
            beta_sb = singles.tile([128, 1], f32)
            nc.sync.dma_start(out=beta_sb[:], in_=beta_p[:].to_broadcast([128, 1]))
            iota_sb = singles.tile([128, 1, 128], bf16)
            nc.sync.dma_start(
                out=iota_sb[:], in_=iota_p[:].to_broadcast([128, 1, 128])
            )
            obuf = singles.tile([128, nw, d], f32)

            with (
                tc.tile_pool(name="inp", bufs=2) as inp,
                tc.tile_pool(name="ap_", bufs=2) as ap_,
                tc.tile_pool(name="med", bufs=2) as med,
                tc.tile_pool(name="sml", bufs=3) as sml,
                tc.tile_pool(name="ps_", bufs=4, space="PSUM") as ps_,
                tc.For_i(0, repeat, 1) if repeat > 1 else nullcontext(),
            ):
                for b in range(nb):
                    ws = slice(b * wb, (b + 1) * wb)
                    fs_t = inp.tile([128, wb, tpw, d], bf16)
                    nc.sync.dma_start(
                        out=fs_t[:], in_=fs_p[ws].rearrange("w i t c -> i w t c")
                    )
                    fd_t = inp.tile([128, wb, tpw, d], bf16)
                    nc.sync.dma_start(
                        out=fd_t[:], in_=fd_p[ws].rearrange("w i t c -> i w t c")
                    )
                    d8_t = inp.tile([128, wb, tpw, 1], bf16)
                    nc.sync.dma_start(
                        out=d8_t[:],
                        in_=d8_p[ws].rearrange("w i t -> i w t")[:, :, :, None],
                    )

                    # one-hot A[e-partition, m] per tile (bf16, exact 0/1)
                    A_t = ap_.tile([128, tb, 128], bf16)
                    nc.vector.tensor_tensor(
                        out=A_t[:],
                        in0=d8_t[:].rearrange("i w t o -> i (w t) o").to_broadcast(
                            [128, tb, 128]
                        ),
                        in1=iota_sb[:].to_broadcast([128, tb, 128]),
                        op=ISEQ,
                    )

                    fsf = fs_t[:].rearrange("i w t c -> i (w t) c")
                    fdf = fd_t[:].rearrange("i w t c -> i (w t) c")

                    # cos numerator (DVE)
                    prod = med.tile([128, tb, d], bf16)
                    nc.vector.tensor_mul(prod[:], fsf, fdf)
                    cos = sml.tile([128, tb, 1], f32)
                    nc.vector.tensor_reduce(cos[:], prod[:], axis=X, op=ADD)
                    # squared norms (Pool engine)
                    sq = med.tile([128, tb, d], bf16)
                    nc.gpsimd.tensor_mul(sq[:], fsf, fsf)
                    ss_s = sml.tile([128, tb, 1], f32)
                    nc.vector.tensor_reduce(ss_s[:], sq[:], axis=X, op=ADD)
                    sq2 = med.tile([128, tb, d], bf16)
                    nc.gpsimd.tensor_mul(sq2[:], fdf, fdf)
                    ss_d = sml.tile([128, tb, 1], f32)
                    nc.vector.tensor_reduce(ss_d[:], sq2[:], axis=X, op=ADD)

                    # rn = 1/sqrt(max(ss_s*ss_d, 1e-24)); cn = cos*rn
                    ssp = sml.tile([128, tb, 1], f32)
                    nc.vector.tensor_mul(ssp[:], ss_s[:], ss_d[:])
                    nc.vector.tensor_scalar_max(ssp[:], ssp[:], 1e-24)
                    nrm = sml.tile([128, tb, 1], f32)
                    nc.scalar.sqrt(nrm[:], ssp[:])
                    rn = sml.tile([128, tb, 1], f32)
                    nc.vector.reciprocal(rn[:], nrm[:])
                    cn = sml.tile([128, tb, 1], f32)
                    nc.vector.tensor_mul(cn[:], cos[:], rn[:])
                    # p = exp(beta*cn)
                    p_t = sml.tile([128, tb, 1], bf16)
                    nc.scalar.activation(
                        p_t[:], cn[:], mybir.ActivationFunctionType.Exp,
                        scale=beta_sb[:],
                    )

                    # payload [p*fs | p] in bf16
                    pay = med.tile([128, tb, PW], bf16)
                    nc.vector.tensor_mul(
                        pay[:, :, 0:d], fsf, p_t[:].to_broadcast([128, tb, d])
                    )
                    nc.vector.tensor_copy(out=pay[:, :, d : d + 1], in_=p_t[:])

                    # scatter: per-window PSUM accumulation over tiles
                    for wj in range(wb):
                        ps = ps_.tile([128, PW], f32)
                        for t in range(tpw):
                            ti = wj * tpw + t
                            nc.tensor.matmul(
                                ps[:],
                                lhsT=A_t[:, ti, :],
                                rhs=pay[:, ti, :],
                                start=(t == 0),
                                stop=(t == tpw - 1),
                            )
                        w = b * wb + wj
                        sc = sml.tile([128, 1], f32)
                        nc.vector.tensor_scalar_max(sc[:], ps[:, d : d + 1], 1e-30)
                        rc = sml.tile([128, 1], f32)
                        nc.vector.reciprocal(rc[:], sc[:])
                        nc.vector.tensor_mul(
                            obuf[:, w, :], ps[:, 0:d], rc[:].to_broadcast([128, d])
                        )

            nc.sync.dma_start(out=outR[:, :, :], in_=obuf[:])

    nc.compile()
    return nc


def host_prep(feat, beta, src, dst, ncores, nw, tpw, d):
    """Pure index/layout prep. Returns per-core input maps."""
    feat = np.ascontiguousarray(np.asarray(feat, dtype=np.float32))
    beta = np.ascontiguousarray(np.asarray(beta, dtype=np.float32))
    src = np.asarray(src).astype(np.int64)
    dst = np.asarray(dst).astype(np.int64)
    n_nodes = feat.shape[0]
    nloc = nw * 128
    cap = tpw * 128

    win = dst // 128                       # global window id
    order = np.argsort(win, kind="stable")
    src_s, dst_s = src[order], dst[order]
    win_s = win[order]
    # rank of each edge within its window
    wcnt = np.bincount(win_s, minlength=ncores * nw)
    assert wcnt.max() <= cap, f"window overflow: {wcnt.max()} > {cap}"
    starts = np.concatenate([[0], np.cumsum(wcnt)[:-1]])
    rank = np.arange(src_s.size) - starts[win_s]

    iota = np.arange(128, dtype=np.float32)[None, None, :]

    in_maps = []
    for c in range(ncores):
        lo_w, hi_w = c * nw, (c + 1) * nw
        sel = (win_s >= lo_w) & (win_s < hi_w)
        e_src, e_dst, e_win, e_rank = (
            src_s[sel], dst_s[sel], win_s[sel] - lo_w, rank[sel],
        )
        t_ = e_rank // 128
        i_ = e_rank % 128

        fs = np.zeros((nw, 128, tpw, d), dtype=np.float32)
        fd = np.zeros((nw, 128, tpw, d), dtype=np.float32)
        d8 = np.full((nw, 128, tpw), 255.0, dtype=np.float32)
        fs[e_win, i_, t_] = feat[e_src]
        fd[e_win, i_, t_] = feat[e_dst]
        d8[e_win, i_, t_] = (e_dst % 128).astype(np.float32)

        in_maps.append(
            {
                "fs": fs,
                "fd": fd,
                "d8": d8,  # f32 values, cast below
                "iota": iota,
                "beta": beta,
            }
        )
    # bf16 conversion via uint32 view trick (values are small ints: exact)
    import ml_dtypes

    for m in in_maps:
        m["d8"] = m["d8"].astype(ml_dtypes.bfloat16)
        m["iota"] = m["iota"].astype(ml_dtypes.bfloat16)
        m["fs"] = m["fs"].astype(ml_dtypes.bfloat16)
        m["fd"] = m["fd"].astype(ml_dtypes.bfloat16)
    return in_maps


_CACHED = {}


def kernel(feat, beta, src, dst):
    from concourse.bass_utils import run_bass_kernel_spmd

    in_maps = host_prep(feat, beta, src, dst, NCORES, NW, TPW, D)
    if "nc" not in _CACHED:
        _CACHED["nc"] = build_graph(NW, TPW, WB)
    nc = _CACHED["nc"]
    res = run_bass_kernel_spmd(nc, in_maps, list(range(NCORES))).results
    full = np.concatenate([res[c]["out"] for c in range(NCORES)], axis=0)
    return full[:N_NODES].astype(np.float32)
